# revision 21
# baseline (speedup 1.0000x reference)
"""Trainium2 Bass kernel for BiochemicalDynamics.

Reference computation (f32):
    Ax    = A @ x                                   # [N, DIM]
    s     = R * rowsum(x * Ax)                      # [N, 1]
    out   = F - B*x - s                             # [N, DIM]

Strategy: row-shard A across the 8 cores (1024 rows each) and compute
A @ x directly on the TensorEngine with A as the *moving* operand in
fp8 (e4m3) and x as the *stationary* operand, producing the transposed
product AxT = (A_loc @ x)^T in PSUM, accumulated over the 64 K-tiles of
the 8192-long contraction.  DoubleRow fp8 perf mode processes two
128-row K-tiles per matmul.

fp8 quantization error on A/x is zero-mean and averages out over the
8192-term reductions (~1.5e-3 relative on the output); the final
rowwise dot s_i = R * <x_i, (Ax)_i> uses a bf16 x, which kills the
error component that would NOT average (the x_i factor).  The output is
returned in bf16 (~2e-3 additional relative error; the harness gate is
2e-2).

Performance notes (from NTFF traces):
 - The A stream (8MB fp8/core) runs at the 16-engine DGE cap
   (~22-24 GB/s per engine, ~345 GB/s/core); everything else hides
   under it or sits in the fixed ~7us framework preamble / ~2us
   teardown windows.
 - DMA issue costs ~0.6us per DMA_DIRECT2D on the issuing engine, so A
   moves in 512KB super-tiles (4KB per-partition lines are the fastest
   observed packet size; 2KB is ~25% slower, 8KB gains nothing) on the
   Sync HWDGE queue, while x/consts/outputs ride the Scalar HWDGE
   queue.  (Alternating A tiles across both queues measured slightly
   worse than a pure single-queue A stream.)
 - The PE HAM clock-gate starts at half rate; ~4us of warmup matmuls
   bring it to full rate just as the first A tile lands.
 - The local 1024 output rows run as two independent 512-column halves:
   half 0's epilogue hides under half 1's A stream, and its reduce
   matmul is deferred past half 1's first two super-tiles so the
   in-order PE queue never stalls on the DVE.  The epilogue is one DVE
   op (tmp = R*xt (.) AxT), two accumulating reduce-matmuls
   (W2 = [diag(-B); F-row] against [xt; ones], W1 = -1s against tmp),
   then a Scalar-engine PSUM->SBUF copy chained (same engine, no
   cross-engine hop) into the output DMA issue.
 - Matmul PSUM writes must stay bank-aligned: a [64, 256] accumulation
   chain at a 1KB column offset inside a bank silently accumulates
   wrong results on TRN2 hardware (verified), so every chain owns a
   full [64, 512] bank.
"""

import sys

import numpy as np

for _p in ("/opt/trn_rl_repo", "/root/.axon_site/_ro/trn_rl_repo"):
    if _p not in sys.path:
        sys.path.append(_p)

N = 8192
DIM = 64
NCORES = 8
ROWS = N // NCORES       # 1024 rows of A per core
RH = 512                 # rows per i-half
F_CONST = 1.0
B_CONST = 0.1
R_CONST = 0.01

P = 128                  # SBUF partitions / K-tile size
KT = N // P              # 64 K-tiles in the contraction
KSUP = 8                 # K-tiles per DMA super-tile (512KB, 4KB lines)
NSUP = KT // KSUP        # 8 super-tiles per i-half
NWARM = 26               # PE warmup matmuls (~160ns each)

_CACHE = {}


def _build_nc():
    import concourse.mybir as mybir
    import concourse.tile as tile
    from concourse import bacc

    f32 = mybir.dt.float32
    bf16 = mybir.dt.bfloat16
    fp8 = mybir.dt.float8e4

    nc = bacc.Bacc(
        trn_type="TRN2", target_bir_lowering=False, debug=False, num_devices=NCORES
    )

    # a: A_loc^T packed per (half, super-tile): a[((h*8+st)*128+p), kk*512+i]
    #    = A[rows_{h*512+i}, ((st*8+kk)*128 + p)]   fp8, [2048, 4096].
    a = nc.dram_tensor("a", [2 * NSUP * P, KSUP * RH], fp8, kind="ExternalInput")
    # xp: x packed for stationary use: xp[p, kt, d] = x[kt*128+p, d], fp8.
    xp = nc.dram_tensor("xp", [P, KT, DIM], fp8, kind="ExternalInput")
    # xt: local x rows transposed, bf16 [64, 1024] (epilogue dot operand).
    xt = nc.dram_tensor("xt", [DIM, ROWS], bf16, kind="ExternalInput")
    # w2: epilogue reduce weights: rows 0..63 = diag(-B), row 64 = F.
    w2d = nc.dram_tensor("w2", [DIM + 1, DIM], bf16, kind="ExternalInput")
    out = nc.dram_tensor("out", [DIM, ROWS], bf16, kind="ExternalOutput")

    mult = mybir.AluOpType.mult
    DR = mybir.MatmulPerfMode.DoubleRow
    Copy = mybir.ActivationFunctionType.Copy

    with tile.TileContext(nc) as tc:
        with (
            tc.tile_pool(name="xpool", bufs=1) as xpool,
            tc.tile_pool(name="apool", bufs=16) as apool,
            tc.tile_pool(name="psum", bufs=1, space="PSUM") as psum_pool,
        ):
            # ---- Scalar-queue DMA: x (first K-tiles lead), consts.
            x_sb = xpool.tile([P, KT, DIM], fp8)
            nc.scalar.dma_start(out=x_sb[:, 0:KSUP, :], in_=xp[:, 0:KSUP, :])

            xv = xpool.tile([DIM + 1, ROWS], bf16, tag="xv")  # [xt; ones]
            w2 = xpool.tile([DIM + 1, DIM], bf16, tag="w2")
            nc.scalar.dma_start(out=xv[0:DIM, :], in_=xt[:])
            nc.scalar.dma_start(out=w2[:], in_=w2d[:])
            nc.scalar.dma_start(out=x_sb[:, KSUP:KT, :], in_=xp[:, KSUP:KT, :])
            nc.vector.memset(xv[DIM : DIM + 1, :], 1.0)

            w1 = xpool.tile([DIM, DIM], bf16, tag="w1")
            nc.vector.memset(w1[:], -1.0)
            tmp = xpool.tile([DIM, ROWS], bf16, tag="tmp")
            o_sb = xpool.tile([DIM, ROWS], bf16, tag="o")

            # ---- PE warmup: keeps the PE busy from the framework preamble
            # until the first A tile lands, lifting the HAM clock gate.
            wlhs = xpool.tile([P, 1], fp8, tag="wlhs")
            wrhs = xpool.tile([P, 256], fp8, tag="wrhs")
            nc.vector.memset(wlhs[:], 0.0)
            nc.vector.memset(wrhs[:], 0.0)
            warm_ps = psum_pool.tile([1, 256], f32, tag="warm")
            for _ in range(NWARM):
                nc.tensor.matmul(
                    warm_ps[:], wlhs[:], wrhs[:], start=True, stop=True,
                )

            ax_ps = [
                psum_pool.tile([DIM, RH], f32, tag=f"ax{h}", name=f"ax{h}")
                for h in range(2)
            ]
            s_ps0 = psum_pool.tile([DIM, RH], f32, tag="s0", name="s0")
            # Half 1's reduce runs as two 256-col pieces pipelined across
            # DVE/PE/Scalar; each piece owns a full bank (matmul PSUM writes
            # at a non-bank-aligned column offset corrupt accumulation).
            s_ps1 = [
                psum_pool.tile([DIM, RH], f32, tag=f"s1{v}", name=f"s1{v}")
                for v in range(2)
            ]

            # A-tile DMAs are all emitted upfront.  Each 512KB super-tile
            # is issued as two 256KB halves, one per HWDGE queue (Sync gets
            # K-tiles 0..3, Scalar gets 4..7): every DGE engine then always
            # has two descriptor streams to pull from, so one queue's
            # descriptor-boundary stall is covered by the other's lines.
            a_tiles = []
            for idx in range(2 * NSUP):
                a_sb = apool.tile([P, KSUP, RH], fp8, tag="a", name=f"a{idx}")
                base = idx * P
                if idx == 2 * NSUP - 1:
                    # Fine-grained last tile: the tail's matmuls wait on as
                    # little of the final transfer as possible.
                    for sp in range(4):
                        eng = nc.sync if sp % 2 == 0 else nc.scalar
                        eng.dma_start(
                            out=a_sb[:, 2 * sp : 2 * sp + 2, :],
                            in_=a[base : base + P, 2 * sp * RH : (2 * sp + 2) * RH],
                        )
                else:
                    nc.sync.dma_start(
                        out=a_sb[:, 0:4, :], in_=a[base : base + P, 0 : 4 * RH]
                    )
                    nc.scalar.dma_start(
                        out=a_sb[:, 4:8, :],
                        in_=a[base : base + P, 4 * RH : 8 * RH],
                    )
                a_tiles.append(a_sb)

            def acc_tile(h, st):
                a_sb = a_tiles[h * NSUP + st]
                for q in range(KSUP // 2):
                    t = st * (KSUP // 2) + q
                    nc.tensor.matmul(
                        ax_ps[h][:],
                        x_sb[:, 2 * t : 2 * t + 2, :],
                        a_sb[:, 2 * q : 2 * q + 2, :],
                        start=(st == 0 and q == 0),
                        stop=(st == NSUP - 1 and q == KSUP // 2 - 1),
                        perf_mode=DR,
                    )

            # ---- Half 0 accumulation.
            for st in range(NSUP):
                acc_tile(0, st)
                if st == 2:
                    # Early parts of the reduce: s_ps = W2^T @ [xt;1]
                    # = -B*xt + F, off the critical tail path.
                    nc.tensor.matmul(
                        s_ps0[:], w2[:], xv[:, 0:RH], start=True, stop=False,
                    )
                    for v in range(2):
                        vs = slice(RH + v * 256, RH + (v + 1) * 256)
                        nc.tensor.matmul(
                            s_ps1[v][:, 0:256], w2[:], xv[:, vs],
                            start=True, stop=False,
                        )

            # Half 0 epilogue DVE op fires as soon as half 0's chain stops.
            h0 = slice(0, RH)
            nc.vector.scalar_tensor_tensor(
                tmp[:, h0], xv[0:DIM, h0], R_CONST, ax_ps[0][:],
                op0=mult, op1=mult,
            )

            # ---- Half 1 accumulation, with half 0's reduce matmul deferred
            # past two super-tiles so the in-order PE queue reaches it after
            # the DVE result is long ready (no PE stall, no HAM dip).
            for st in range(NSUP):
                acc_tile(1, st)
                if st == 1:
                    nc.tensor.matmul(
                        s_ps0[:], w1[:], tmp[:, h0], start=False, stop=True,
                    )
                    nc.scalar.activation(o_sb[:, h0], s_ps0[:], Copy)
                    nc.scalar.dma_start(out=out[:, h0], in_=o_sb[:, h0])

            # ---- Half 1 epilogue (the tail): two 256-col pieces pipeline
            # across DVE -> PE -> Scalar with ~40ns sem hops; the store is
            # issued from the Sync queue, still hot from the A stream (a
            # cold HWDGE queue adds ~1us of descriptor-fetch latency).
            for v in range(2):
                vs = slice(RH + v * 256, RH + (v + 1) * 256)
                nc.vector.scalar_tensor_tensor(
                    tmp[:, vs], xv[0:DIM, vs], R_CONST,
                    ax_ps[1][:, v * 256 : (v + 1) * 256],
                    op0=mult, op1=mult,
                )
            for v in range(2):
                vs = slice(RH + v * 256, RH + (v + 1) * 256)
                nc.tensor.matmul(
                    s_ps1[v][:, 0:256], w1[:], tmp[:, vs],
                    start=False, stop=True,
                )
            for v in range(2):
                vs = slice(RH + v * 256, RH + (v + 1) * 256)
                nc.scalar.activation(o_sb[:, vs], s_ps1[v][:, 0:256], Copy)
            nc.sync.dma_start(
                out=out[:, RH : 2 * RH], in_=o_sb[:, RH : 2 * RH]
            )

    nc.finalize()
    return nc


def _get_nc():
    if "nc" not in _CACHE:
        _CACHE["nc"] = _build_nc()
    return _CACHE["nc"]


def _make_in_maps(x, A):
    import ml_dtypes

    e4 = ml_dtypes.float8_e4m3
    bf = ml_dtypes.bfloat16
    x = np.ascontiguousarray(np.asarray(x, dtype=np.float32))
    A = np.asarray(A, dtype=np.float32)

    x8 = x.astype(e4)
    # xp[p, kt, d] = x[kt*128 + p, d]
    xp = np.ascontiguousarray(x8.reshape(KT, P, DIM).transpose(1, 0, 2))
    A8 = A.astype(e4)

    w2 = np.zeros((DIM + 1, DIM), dtype=np.float32)
    w2[np.arange(DIM), np.arange(DIM)] = -B_CONST
    w2[DIM, :] = F_CONST
    w2 = w2.astype(bf)

    in_maps = []
    for c in range(NCORES):
        rows = slice(c * ROWS, (c + 1) * ROWS)
        ATc = A8[rows].T  # [8192 j, 1024 i]
        halves = []
        for h in range(2):
            Ah = ATc[:, h * RH : (h + 1) * RH]
            # [st, kk, p, i] -> [st, p, kk, i]
            halves.append(
                np.ascontiguousarray(
                    Ah.reshape(NSUP, KSUP, P, RH).transpose(0, 2, 1, 3)
                )
            )
        at = np.concatenate(halves).reshape(2 * NSUP * P, KSUP * RH)
        in_maps.append(
            {
                "a": at,
                "xp": xp,
                "xt": np.ascontiguousarray(x[rows].T).astype(bf),
                "w2": w2,
            }
        )
    return in_maps


def run_sharded(x, A, trace=False, **kwargs):
    """Run the SPMD bass kernel; returns (full_output, BassKernelResults)."""
    from concourse.bass_utils import run_bass_kernel_spmd

    nc = _get_nc()
    res = run_bass_kernel_spmd(
        nc, _make_in_maps(x, A), core_ids=list(range(NCORES)), trace=trace, **kwargs
    )
    full = np.concatenate(
        [
            np.ascontiguousarray(res.results[c]["out"].astype(np.float32).T)
            for c in range(NCORES)
        ],
        axis=0,
    )
    return full.astype(np.float32, copy=False), res


def kernel(t, x, A):
    out, _ = run_sharded(x, A)
    return out


# revision 27
# speedup vs baseline: 1.0442x; 1.0442x over previous
"""Trainium2 Bass kernel for BiochemicalDynamics.

Reference computation (f32):
    Ax    = A @ x                                   # [N, DIM]
    s     = R * rowsum(x * Ax)                      # [N, 1]
    out   = F - B*x - s                             # [N, DIM]

Strategy: row-shard A across the 8 cores (1024 rows each) and compute
A @ x directly on the TensorEngine with A as the *moving* operand in
fp8 (e4m3) and x as the *stationary* operand, producing the transposed
product AxT = (A_loc @ x)^T in PSUM, accumulated over the 64 K-tiles of
the 8192-long contraction.  DoubleRow fp8 perf mode processes two
128-row K-tiles per matmul.

fp8 quantization error on A/x is zero-mean and averages out over the
8192-term reductions (~1.5e-3 relative on the output); the final
rowwise dot s_i = R * <x_i, (Ax)_i> uses a bf16 x, which kills the
error component that would NOT average (the x_i factor).  The output is
returned in bf16 (~2e-3 additional relative error; the harness gate is
2e-2).

Performance notes (from NTFF traces):
 - The A stream (8MB fp8/core) runs at the 16-engine DGE cap
   (~22-24 GB/s per engine, ~345 GB/s/core); everything else hides
   under it or sits in the fixed ~7us framework preamble / ~2us
   teardown windows.
 - DMA issue costs ~0.6us per DMA_DIRECT2D on the issuing engine, so A
   moves in 512KB super-tiles (4KB per-partition lines are the fastest
   observed packet size; 2KB is ~25% slower, 8KB gains nothing) on the
   Sync HWDGE queue, while x/consts/outputs ride the Scalar HWDGE
   queue.  (Alternating A tiles across both queues measured slightly
   worse than a pure single-queue A stream.)
 - The PE HAM clock-gate starts at half rate; ~4us of warmup matmuls
   bring it to full rate just as the first A tile lands.
 - The local 1024 output rows run as two independent 512-column halves:
   half 0's epilogue hides under half 1's A stream, and its reduce
   matmul is deferred past half 1's first two super-tiles so the
   in-order PE queue never stalls on the DVE.  The epilogue is one DVE
   op (tmp = R*xt (.) AxT), two accumulating reduce-matmuls
   (W2 = [diag(-B); F-row] against [xt; ones], W1 = -1s against tmp),
   then a Scalar-engine PSUM->SBUF copy chained (same engine, no
   cross-engine hop) into the output DMA issue.
 - Matmul PSUM writes must stay bank-aligned: a [64, 256] accumulation
   chain at a 1KB column offset inside a bank silently accumulates
   wrong results on TRN2 hardware (verified), so every chain owns a
   full [64, 512] bank.
"""

import sys

import numpy as np

for _p in ("/opt/trn_rl_repo", "/root/.axon_site/_ro/trn_rl_repo"):
    if _p not in sys.path:
        sys.path.append(_p)

N = 8192
DIM = 64
NCORES = 8
ROWS = N // NCORES       # 1024 rows of A per core
RH = 512                 # rows per i-half
F_CONST = 1.0
B_CONST = 0.1
R_CONST = 0.01

P = 128                  # SBUF partitions / K-tile size
KT = N // P              # 64 K-tiles in the contraction
KSUP = 8                 # K-tiles per DMA super-tile (512KB, 4KB lines)
NSUP = KT // KSUP        # 8 super-tiles per i-half
NWARM = 26               # PE warmup matmuls (~160ns each)

_CACHE = {}


def _build_nc():
    import concourse.mybir as mybir
    import concourse.tile as tile
    from concourse import bacc

    f32 = mybir.dt.float32
    bf16 = mybir.dt.bfloat16
    fp8 = mybir.dt.float8e4

    nc = bacc.Bacc(
        trn_type="TRN2", target_bir_lowering=False, debug=False, num_devices=NCORES
    )

    # a: A_loc^T packed per (half, super-tile): a[((h*8+st)*128+p), kk*512+i]
    #    = A[rows_{h*512+i}, ((st*8+kk)*128 + p)]   fp8, [2048, 4096].
    a = nc.dram_tensor("a", [2 * NSUP * P, KSUP * RH], fp8, kind="ExternalInput")
    # xp: x packed for stationary use: xp[p, kt, d] = x[kt*128+p, d], fp8.
    xp = nc.dram_tensor("xp", [P, KT, DIM], fp8, kind="ExternalInput")
    # xt: local x rows transposed, bf16 [64, 1024] (epilogue dot operand).
    xt = nc.dram_tensor("xt", [DIM, ROWS], bf16, kind="ExternalInput")
    # w2: epilogue reduce weights: rows 0..63 = diag(-B), row 64 = F.
    w2d = nc.dram_tensor("w2", [DIM + 1, DIM], bf16, kind="ExternalInput")
    out = nc.dram_tensor("out", [DIM, ROWS], bf16, kind="ExternalOutput")

    mult = mybir.AluOpType.mult
    DR = mybir.MatmulPerfMode.DoubleRow
    Copy = mybir.ActivationFunctionType.Copy

    with tile.TileContext(nc) as tc:
        with (
            tc.tile_pool(name="xpool", bufs=1) as xpool,
            tc.tile_pool(name="apool", bufs=16) as apool,
            tc.tile_pool(name="psum", bufs=1, space="PSUM") as psum_pool,
        ):
            # ---- Scalar-queue DMA: x (first K-tiles lead), consts.
            x_sb = xpool.tile([P, KT, DIM], fp8)
            nc.scalar.dma_start(out=x_sb[:, 0:KSUP, :], in_=xp[:, 0:KSUP, :])

            xv = xpool.tile([DIM + 1, ROWS], bf16, tag="xv")  # [xt; ones]
            w2 = xpool.tile([DIM + 1, DIM], bf16, tag="w2")
            nc.scalar.dma_start(out=xv[0:DIM, :], in_=xt[:])
            nc.scalar.dma_start(out=w2[:], in_=w2d[:])
            nc.scalar.dma_start(out=x_sb[:, KSUP:KT, :], in_=xp[:, KSUP:KT, :])
            nc.vector.memset(xv[DIM : DIM + 1, :], 1.0)

            w1 = xpool.tile([DIM, DIM], bf16, tag="w1")
            nc.vector.memset(w1[:], -1.0)
            tmp = xpool.tile([DIM, ROWS], bf16, tag="tmp")
            o_sb = xpool.tile([DIM, ROWS], bf16, tag="o")

            # ---- PE warmup: keeps the PE busy from the framework preamble
            # until the first A tile lands, lifting the HAM clock gate.
            wlhs = xpool.tile([P, 1], fp8, tag="wlhs")
            wrhs = xpool.tile([P, 256], fp8, tag="wrhs")
            nc.vector.memset(wlhs[:], 0.0)
            nc.vector.memset(wrhs[:], 0.0)
            warm_ps = psum_pool.tile([1, 256], f32, tag="warm")
            for _ in range(NWARM):
                nc.tensor.matmul(
                    warm_ps[:], wlhs[:], wrhs[:], start=True, stop=True,
                )

            ax_ps = [
                psum_pool.tile([DIM, RH], f32, tag=f"ax{h}", name=f"ax{h}")
                for h in range(2)
            ]
            s_ps0 = psum_pool.tile([DIM, RH], f32, tag="s0", name="s0")
            # Half 1's reduce runs as two 256-col pieces pipelined across
            # DVE/PE/Scalar; each piece owns a full bank (matmul PSUM writes
            # at a non-bank-aligned column offset corrupt accumulation).
            s_ps1 = [
                psum_pool.tile([DIM, RH], f32, tag=f"s1{v}", name=f"s1{v}")
                for v in range(2)
            ]

            # A-tile DMAs are all emitted upfront on the Sync queue; the
            # tile pool's buffer-reuse WAR dependencies gate issue depth.
            # (Splitting tiles across both HWDGE queues measured ~1.5us
            # MORE DMA busy time: extra descriptors add boundaries.)
            a_tiles = []
            for idx in range(2 * NSUP):
                a_sb = apool.tile([P, KSUP, RH], fp8, tag="a", name=f"a{idx}")
                base = idx * P
                eng = nc.sync
                if idx == 0:
                    eng.dma_start(
                        out=a_sb[:, 0:4, :], in_=a[base : base + P, 0 : 4 * RH]
                    )
                    eng.dma_start(
                        out=a_sb[:, 4:8, :],
                        in_=a[base : base + P, 4 * RH : 8 * RH],
                    )
                elif idx == 2 * NSUP - 1:
                    # Fine-grained last tile: the tail's matmuls wait on as
                    # little of the final transfer as possible.
                    for sp in range(4):
                        eng.dma_start(
                            out=a_sb[:, 2 * sp : 2 * sp + 2, :],
                            in_=a[base : base + P, 2 * sp * RH : (2 * sp + 2) * RH],
                        )
                else:
                    eng.dma_start(out=a_sb[:], in_=a[base : base + P, :])
                a_tiles.append(a_sb)

            def acc_tile(h, st):
                a_sb = a_tiles[h * NSUP + st]
                for q in range(KSUP // 2):
                    t = st * (KSUP // 2) + q
                    nc.tensor.matmul(
                        ax_ps[h][:],
                        x_sb[:, 2 * t : 2 * t + 2, :],
                        a_sb[:, 2 * q : 2 * q + 2, :],
                        start=(st == 0 and q == 0),
                        stop=(st == NSUP - 1 and q == KSUP // 2 - 1),
                        perf_mode=DR,
                    )

            # ---- Half 0 accumulation.
            for st in range(NSUP):
                acc_tile(0, st)
                if st == 2:
                    # Early parts of the reduce: s_ps = W2^T @ [xt;1]
                    # = -B*xt + F, off the critical tail path.
                    nc.tensor.matmul(
                        s_ps0[:], w2[:], xv[:, 0:RH], start=True, stop=False,
                    )
                    for v in range(2):
                        vs = slice(RH + v * 256, RH + (v + 1) * 256)
                        nc.tensor.matmul(
                            s_ps1[v][:, 0:256], w2[:], xv[:, vs],
                            start=True, stop=False,
                        )

            # Half 0 epilogue DVE op fires as soon as half 0's chain stops.
            h0 = slice(0, RH)
            nc.vector.scalar_tensor_tensor(
                tmp[:, h0], xv[0:DIM, h0], R_CONST, ax_ps[0][:],
                op0=mult, op1=mult,
            )

            # ---- Half 1 accumulation, with half 0's reduce matmul deferred
            # past two super-tiles so the in-order PE queue reaches it after
            # the DVE result is long ready (no PE stall, no HAM dip).
            for st in range(NSUP):
                acc_tile(1, st)
                if st == 1:
                    nc.tensor.matmul(
                        s_ps0[:], w1[:], tmp[:, h0], start=False, stop=True,
                    )
                    nc.scalar.activation(o_sb[:, h0], s_ps0[:], Copy)
                    nc.scalar.dma_start(out=out[:, h0], in_=o_sb[:, h0])

            # ---- Half 1 epilogue (the tail): two 256-col pieces pipeline
            # across DVE -> PE -> Scalar with ~40ns sem hops; the store is
            # issued from the Sync queue, still hot from the A stream (a
            # cold HWDGE queue adds ~1us of descriptor-fetch latency).
            for v in range(2):
                vs = slice(RH + v * 256, RH + (v + 1) * 256)
                nc.vector.scalar_tensor_tensor(
                    tmp[:, vs], xv[0:DIM, vs], R_CONST,
                    ax_ps[1][:, v * 256 : (v + 1) * 256],
                    op0=mult, op1=mult,
                )
            for v in range(2):
                vs = slice(RH + v * 256, RH + (v + 1) * 256)
                nc.tensor.matmul(
                    s_ps1[v][:, 0:256], w1[:], tmp[:, vs],
                    start=False, stop=True,
                )
            for v in range(2):
                vs = slice(RH + v * 256, RH + (v + 1) * 256)
                nc.scalar.activation(o_sb[:, vs], s_ps1[v][:, 0:256], Copy)
            nc.sync.dma_start(
                out=out[:, RH : 2 * RH], in_=o_sb[:, RH : 2 * RH]
            )

    nc.finalize()
    return nc


def _get_nc():
    if "nc" not in _CACHE:
        _CACHE["nc"] = _build_nc()
    return _CACHE["nc"]


def _make_in_maps(x, A):
    import ml_dtypes

    e4 = ml_dtypes.float8_e4m3
    bf = ml_dtypes.bfloat16
    x = np.ascontiguousarray(np.asarray(x, dtype=np.float32))
    A = np.asarray(A, dtype=np.float32)

    x8 = x.astype(e4)
    # xp[p, kt, d] = x[kt*128 + p, d]
    xp = np.ascontiguousarray(x8.reshape(KT, P, DIM).transpose(1, 0, 2))
    A8 = A.astype(e4)

    w2 = np.zeros((DIM + 1, DIM), dtype=np.float32)
    w2[np.arange(DIM), np.arange(DIM)] = -B_CONST
    w2[DIM, :] = F_CONST
    w2 = w2.astype(bf)

    in_maps = []
    for c in range(NCORES):
        rows = slice(c * ROWS, (c + 1) * ROWS)
        ATc = A8[rows].T  # [8192 j, 1024 i]
        halves = []
        for h in range(2):
            Ah = ATc[:, h * RH : (h + 1) * RH]
            # [st, kk, p, i] -> [st, p, kk, i]
            halves.append(
                np.ascontiguousarray(
                    Ah.reshape(NSUP, KSUP, P, RH).transpose(0, 2, 1, 3)
                )
            )
        at = np.concatenate(halves).reshape(2 * NSUP * P, KSUP * RH)
        in_maps.append(
            {
                "a": at,
                "xp": xp,
                "xt": np.ascontiguousarray(x[rows].T).astype(bf),
                "w2": w2,
            }
        )
    return in_maps


def run_sharded(x, A, trace=False, **kwargs):
    """Run the SPMD bass kernel; returns (full_output, BassKernelResults)."""
    from concourse.bass_utils import run_bass_kernel_spmd

    nc = _get_nc()
    res = run_bass_kernel_spmd(
        nc, _make_in_maps(x, A), core_ids=list(range(NCORES)), trace=trace, **kwargs
    )
    full = np.concatenate(
        [
            np.ascontiguousarray(res.results[c]["out"].astype(np.float32).T)
            for c in range(NCORES)
        ],
        axis=0,
    )
    return full.astype(np.float32, copy=False), res


def kernel(t, x, A):
    out, _ = run_sharded(x, A)
    return out
